# revision 1
# baseline (speedup 1.0000x reference)
"""Grouped SwiGLU expert FFN (MoE) on 8 Trainium2 NeuronCores.

Expert parallelism: expert e's weights + its (pre-sorted) token slice go to
core e. Each core runs x@w1, x@w3, silu/mul, h@w2 for its 8192 tokens.

Math per core (dims: t=tokens, i=dim_in, j=dim_hid, o=dim_in):
  mm1/mm3: psum[j,t] += w{1,3}[i,j].T-style: lhsT=w[i_chunk, j_chunk] (natural
           layout, stationary), rhs=xT[i_chunk, t_block] (moving) -> h1T/h3T.
           Requires x transposed; done on-chip via PE transpose.
  SwiGLU:  hT = silu(h1T) * h3T  (ACT Silu + DVE mul, PSUM eviction fused).
  mm2:     lhsT=hT[j_chunk, t_chunk] (stationary), rhs=w2[j_chunk, o_block]
           (moving) -> psum[t,o] = natural-layout output. No output transpose.

All matmuls run in float32r (full PE rate at moving dim >= 256, ~1.5e-4 rel
err vs 2.3e-3 for bf16 -- measured on HW).
"""

import sys

sys.path.insert(0, "/opt/trn_rl_repo")

import numpy as np

N_CORES = 8
D = 1024  # dim_in
H = 1024  # dim_hid
P = 128
TB = 256  # token block per pipeline stage

_CACHE = {}


def _build(tok):
    import concourse.bacc as bacc
    import concourse.tile as tile
    from concourse import mybir
    from concourse.masks import make_identity

    dt = mybir.dt
    AF = mybir.ActivationFunctionType
    f32 = dt.float32
    f32r = dt.float32r

    assert tok % TB == 0
    n_blk = tok // TB
    n_i = D // P   # 8 contraction chunks for mm1/mm3
    n_j = H // P   # 8 contraction chunks for mm2
    n_tc = TB // P  # 2 token chunks per block
    n_o = D // 512  # 2 output column blocks

    nc = bacc.Bacc(trn_type="TRN2", target_bir_lowering=False)
    x_h = nc.dram_tensor("x", [tok, D], f32, kind="ExternalInput")
    w1_h = nc.dram_tensor("w1", [D, H], f32, kind="ExternalInput")
    w2_h = nc.dram_tensor("w2", [H, D], f32, kind="ExternalInput")
    w3_h = nc.dram_tensor("w3", [D, H], f32, kind="ExternalInput")
    out_h = nc.dram_tensor("out", [tok, D], f32, kind="ExternalOutput")

    with tile.TileContext(nc) as tc:
        with (
            tc.tile_pool(name="wpool", bufs=1) as wpool,
            tc.tile_pool(name="const", bufs=1) as const,
            tc.tile_pool(name="xpool", bufs=2) as xpool,
            tc.tile_pool(name="xtpool", bufs=2) as xtpool,
            tc.tile_pool(name="htpool", bufs=2) as htpool,
            tc.tile_pool(name="spool", bufs=3) as spool,
            tc.tile_pool(name="opool", bufs=2) as opool,
            tc.tile_pool(name="pT", bufs=2, space="PSUM") as pTp,
            tc.tile_pool(name="pA", bufs=2, space="PSUM") as pAp,
            tc.tile_pool(name="pB", bufs=2, space="PSUM") as pBp,
            tc.tile_pool(name="pC", bufs=2, space="PSUM") as pCp,
        ):
            ident = const.tile([P, P], f32)
            make_identity(nc, ident)

            # Resident weights, partition = row-within-chunk: [P, n_chunks, cols]
            w1s = wpool.tile([P, n_i, H], f32r)
            w3s = wpool.tile([P, n_i, H], f32r)
            w2s = wpool.tile([P, n_j, D], f32r)
            nc.sync.dma_start(
                out=w1s, in_=w1_h[:, :].rearrange("(c p) h -> p c h", p=P).bitcast(f32r)
            )
            nc.sync.dma_start(
                out=w3s, in_=w3_h[:, :].rearrange("(c p) h -> p c h", p=P).bitcast(f32r)
            )
            nc.sync.dma_start(
                out=w2s, in_=w2_h[:, :].rearrange("(c p) h -> p c h", p=P).bitcast(f32r)
            )

            x_r = x_h[:, :].rearrange("(b c p) d -> b p c d", p=P, c=n_tc)
            o_r = out_h[:, :].rearrange("(b c p) d -> b p c d", p=P, c=n_tc)

            for b in range(n_blk):
                # ---- load x block, natural layout [P, n_tc, D]
                x_sb = xpool.tile([P, n_tc, D], f32)
                nc.sync.dma_start(out=x_sb, in_=x_r[b])

                # ---- PE-transpose into xT [P(=i in chunk), n_i, TB] f32r
                xT = xtpool.tile([P, n_i, TB], f32r)
                for t in range(n_tc):
                    for i in range(n_i):
                        pT = pTp.tile([P, P], f32)
                        nc.tensor.transpose(
                            pT, x_sb[:, t, i * P:(i + 1) * P], ident
                        )
                        nc.scalar.activation(
                            xT[:, i, t * P:(t + 1) * P], pT, AF.Copy
                        )

                # ---- mm1/mm3 + SwiGLU -> hT [P(=j in chunk), n_j, TB] f32r
                hT = htpool.tile([P, n_j, TB], f32r)
                for j in range(n_j):
                    pA = pAp.tile([P, TB], f32)
                    pB = pBp.tile([P, TB], f32)
                    for i in range(n_i):
                        nc.tensor.matmul(
                            pA, w1s[:, i, j * P:(j + 1) * P], xT[:, i, :],
                            start=(i == 0), stop=(i == n_i - 1),
                        )
                    for i in range(n_i):
                        nc.tensor.matmul(
                            pB, w3s[:, i, j * P:(j + 1) * P], xT[:, i, :],
                            start=(i == 0), stop=(i == n_i - 1),
                        )
                    s1 = spool.tile([P, TB], f32)
                    nc.scalar.activation(s1, pA, AF.Silu)
                    nc.vector.tensor_mul(hT[:, j, :], pB, s1)

                # ---- mm2 -> natural-layout out block
                o_sb = opool.tile([P, n_tc, D], f32)
                for t in range(n_tc):
                    for o in range(n_o):
                        pC = pCp.tile([P, 512], f32)
                        for j in range(n_j):
                            nc.tensor.matmul(
                                pC,
                                hT[:, j, t * P:(t + 1) * P],
                                w2s[:, j, o * 512:(o + 1) * 512],
                                start=(j == 0), stop=(j == n_j - 1),
                            )
                        nc.scalar.activation(
                            o_sb[:, t, o * 512:(o + 1) * 512], pC, AF.Copy
                        )
                nc.sync.dma_start(out=o_r[b], in_=o_sb)

    nc.compile()
    return nc


def _get_nc(tok):
    if tok not in _CACHE:
        _CACHE[tok] = _build(tok)
    return _CACHE[tok]


def kernel(x, w1, w2, w3, m_sizes):
    from concourse.bass_utils import run_bass_kernel_spmd

    x = np.asarray(x, dtype=np.float32)
    w1 = np.asarray(w1, dtype=np.float32)
    w2 = np.asarray(w2, dtype=np.float32)
    w3 = np.asarray(w3, dtype=np.float32)
    sizes = np.asarray(m_sizes).astype(np.int64)
    offs = np.concatenate([[0], np.cumsum(sizes)])
    n_exp = sizes.shape[0]
    assert n_exp == N_CORES

    pad = int(max(int(sizes.max()), TB))
    pad = ((pad + TB - 1) // TB) * TB
    nc = _get_nc(pad)

    in_maps = []
    for e in range(N_CORES):
        xe = x[offs[e]:offs[e + 1]]
        if xe.shape[0] < pad:
            xe = np.concatenate(
                [xe, np.zeros((pad - xe.shape[0], D), dtype=np.float32)], axis=0
            )
        in_maps.append({"x": xe, "w1": w1[e], "w2": w2[e], "w3": w3[e]})

    r = run_bass_kernel_spmd(nc, in_maps, core_ids=list(range(N_CORES)))
    out = np.concatenate(
        [r.results[e]["out"][: sizes[e]] for e in range(N_CORES)], axis=0
    )
    return out.astype(np.float32)



# revision 3
# speedup vs baseline: 1.2641x; 1.2641x over previous
"""Grouped SwiGLU expert FFN (MoE) on 8 Trainium2 NeuronCores.

Expert parallelism: expert e's weights + its (pre-sorted) token slice go to
core e. Each core runs x@w1, x@w3, silu/mul, h@w2 for its 8192 tokens.

Math per core (dims: t=tokens, i=dim_in, j=dim_hid, o=dim_in):
  mm1/mm3: psum[j,t] += lhsT=w{1,3}[i_chunk, j_chunk] (stationary, natural
           layout), rhs=xT[i_chunk, t_block] (moving, 512 wide) -> h1T/h3T.
           x transposed on-chip via PE transpose, 4 transposes packed per
           PSUM bank, evicted as one [128,512] copy (scalar/vector alternate).
  SwiGLU:  hT = silu(h1T) * h3T  (ACT Silu -> s1, DVE mul fuses PSUM evict).
  mm2:     lhsT=hT[j_chunk, t_chunk] (stationary), rhs=w2[j_chunk, o_half]
           (moving, 512 wide) -> psum[t,o] natural-layout output.

All matmuls in float32r (full PE rate at moving dim 512, ~2.5e-4 rel err).
A single 8-slot PSUM pool rotates banks through transpose groups, mm1/mm3
accumulators and mm2 accumulators, so all 8 banks stay in flight.
The w1 j-loop runs before the w3 j-loop so the first block only waits on
the w1 DMA (issued right after x block 0, before w3/w2).
"""

import sys

sys.path.insert(0, "/opt/trn_rl_repo")

import numpy as np

N_CORES = 8
D = 1024  # dim_in
H = 1024  # dim_hid
P = 128
TB = 512  # token block per pipeline stage

_CACHE = {}


def _build(tok):
    import concourse.bacc as bacc
    import concourse.tile as tile
    from concourse import mybir
    from concourse.masks import make_identity

    dt = mybir.dt
    AF = mybir.ActivationFunctionType
    f32 = dt.float32
    f32r = dt.float32r

    assert tok % TB == 0
    n_blk = tok // TB
    n_tc = TB // P  # 4 token chunks of 128 per block
    n_i = D // P    # 8 contraction chunks for mm1/mm3
    n_j = H // P    # 8 contraction chunks for mm2

    nc = bacc.Bacc(trn_type="TRN2", target_bir_lowering=False)
    x_h = nc.dram_tensor("x", [tok, D], f32, kind="ExternalInput")
    w1_h = nc.dram_tensor("w1", [D, H], f32, kind="ExternalInput")
    w2_h = nc.dram_tensor("w2", [H, D], f32, kind="ExternalInput")
    w3_h = nc.dram_tensor("w3", [D, H], f32, kind="ExternalInput")
    out_h = nc.dram_tensor("out", [tok, D], f32, kind="ExternalOutput")

    with tile.TileContext(nc) as tc:
        with (
            tc.tile_pool(name="wpool", bufs=1) as wpool,
            tc.tile_pool(name="const", bufs=1) as const,
            tc.tile_pool(name="xpool", bufs=5) as xpool,
            tc.tile_pool(name="xtpool", bufs=2) as xtpool,
            tc.tile_pool(name="htpool", bufs=2) as htpool,
            tc.tile_pool(name="spool", bufs=8) as spool,
            tc.tile_pool(name="opool", bufs=2) as opool,
            tc.tile_pool(name="psum", bufs=8, space="PSUM") as pp,
        ):
            ident = const.tile([P, P], f32)
            make_identity(nc, ident)

            x_r = x_h[:, :].rearrange("(b c p) d -> b c p d", p=P, c=n_tc)
            o_r = out_h[:, :].rearrange("(b c p) d -> b c p d", p=P, c=n_tc)

            x_tiles = {}

            def load_x_block(b):
                tiles = []
                for t in range(n_tc):
                    xt = xpool.tile([P, D], f32, name="xt", tag="xt")
                    nc.sync.dma_start(out=xt, in_=x_r[b, t])
                    tiles.append(xt)
                x_tiles[b] = tiles

            # x block 0 first, then w1 (needed first), then w3, then w2.
            load_x_block(0)
            w1c = [wpool.tile([P, H], f32r, name=f"w1c{i}") for i in range(n_i)]
            w3c = [wpool.tile([P, H], f32r, name=f"w3c{i}") for i in range(n_i)]
            w2c = [wpool.tile([P, D], f32r, name=f"w2c{j}") for j in range(n_j)]
            for i in range(n_i):
                nc.sync.dma_start(
                    out=w1c[i], in_=w1_h[i * P:(i + 1) * P, :].bitcast(f32r)
                )
            for i in range(n_i):
                nc.sync.dma_start(
                    out=w3c[i], in_=w3_h[i * P:(i + 1) * P, :].bitcast(f32r)
                )
            for j in range(n_j):
                nc.sync.dma_start(
                    out=w2c[j], in_=w2_h[j * P:(j + 1) * P, :].bitcast(f32r)
                )

            for b in range(n_blk):
                if b + 1 < n_blk:
                    load_x_block(b + 1)
                xts = x_tiles.pop(b)

                # ---- transposes: per i-chunk, 4 transposes into one PSUM
                # bank, one [P, TB] eviction (scalar/vector alternating).
                xT = xtpool.tile([P, n_i, TB], f32r, name="xT")
                for i in range(n_i):
                    pT = pp.tile([P, TB], f32, name="pT", tag="ps")
                    for t in range(n_tc):
                        nc.tensor.transpose(
                            pT[:, t * P:(t + 1) * P],
                            xts[t][:, i * P:(i + 1) * P],
                            ident,
                        )
                    if i % 2 == 0:
                        nc.scalar.activation(xT[:, i, :], pT, AF.Copy)
                    else:
                        nc.vector.tensor_copy(xT[:, i, :], pT)

                # ---- mm1 (j loop; only needs w1) -> s1[j] = silu(x @ w1)
                s1s = []
                for j in range(n_j):
                    pA = pp.tile([P, TB], f32, name="pA", tag="ps")
                    for i in range(n_i):
                        nc.tensor.matmul(
                            pA, w1c[i][:, j * P:(j + 1) * P], xT[:, i, :],
                            start=(i == 0), stop=(i == n_i - 1),
                        )
                    s1 = spool.tile([P, TB], f32, name="s1", tag="s1")
                    nc.scalar.activation(s1, pA, AF.Silu)
                    s1s.append(s1)

                # ---- mm3 (j loop; needs w3) -> hT[j] = s1[j] * (x @ w3)
                hT = htpool.tile([P, n_j, TB], f32r, name="hT")
                for j in range(n_j):
                    pB = pp.tile([P, TB], f32, name="pB", tag="ps")
                    for i in range(n_i):
                        nc.tensor.matmul(
                            pB, w3c[i][:, j * P:(j + 1) * P], xT[:, i, :],
                            start=(i == 0), stop=(i == n_i - 1),
                        )
                    nc.vector.tensor_mul(hT[:, j, :], pB, s1s[j])

                # ---- mm2 -> natural-layout out, two 512-col halves
                for t in range(n_tc):
                    pC = pp.tile([P, 512], f32, name="pC", tag="ps")
                    for j in range(n_j):
                        nc.tensor.matmul(
                            pC, hT[:, j, t * P:(t + 1) * P], w2c[j][:, 0:512],
                            start=(j == 0), stop=(j == n_j - 1),
                        )
                    pD = pp.tile([P, 512], f32, name="pD", tag="ps")
                    for j in range(n_j):
                        nc.tensor.matmul(
                            pD, hT[:, j, t * P:(t + 1) * P], w2c[j][:, 512:1024],
                            start=(j == 0), stop=(j == n_j - 1),
                        )
                    o_t = opool.tile([P, D], f32, name="o_t", tag="o_t")
                    nc.scalar.activation(o_t[:, 0:512], pC, AF.Copy)
                    nc.vector.tensor_copy(o_t[:, 512:1024], pD)
                    nc.sync.dma_start(out=o_r[b, t], in_=o_t)

    nc.compile()
    return nc


def _get_nc(tok):
    if tok not in _CACHE:
        _CACHE[tok] = _build(tok)
    return _CACHE[tok]


def kernel(x, w1, w2, w3, m_sizes):
    from concourse.bass_utils import run_bass_kernel_spmd

    x = np.asarray(x, dtype=np.float32)
    w1 = np.asarray(w1, dtype=np.float32)
    w2 = np.asarray(w2, dtype=np.float32)
    w3 = np.asarray(w3, dtype=np.float32)
    sizes = np.asarray(m_sizes).astype(np.int64)
    offs = np.concatenate([[0], np.cumsum(sizes)])
    n_exp = sizes.shape[0]
    assert n_exp == N_CORES

    pad = int(max(int(sizes.max()), TB))
    pad = ((pad + TB - 1) // TB) * TB
    nc = _get_nc(pad)

    in_maps = []
    for e in range(N_CORES):
        xe = x[offs[e]:offs[e + 1]]
        if xe.shape[0] < pad:
            xe = np.concatenate(
                [xe, np.zeros((pad - xe.shape[0], D), dtype=np.float32)], axis=0
            )
        in_maps.append({"x": xe, "w1": w1[e], "w2": w2[e], "w3": w3[e]})

    r = run_bass_kernel_spmd(nc, in_maps, core_ids=list(range(N_CORES)))
    out = np.concatenate(
        [r.results[e]["out"][: sizes[e]] for e in range(N_CORES)], axis=0
    )
    return out.astype(np.float32)


# revision 9
# speedup vs baseline: 1.3439x; 1.0632x over previous
"""Grouped SwiGLU expert FFN (MoE) on 8 Trainium2 NeuronCores.

Expert parallelism: expert e's weights + its (pre-sorted) token slice go to
core e. Each core runs x@w1, x@w3, silu/mul, h@w2 for its 8192 tokens.

All matmul operands are converted to bf16 on-chip (PSUM accumulation stays
fp32; bf16 runs the PE at the same 1 row/cycle as f32r). The x transpose is
done by the DMA XBAR (one [128, 4096] bf16 SBUF->SBUF transpose per 512-token
block), so the PE runs matmuls only.

Math per core (dims: t=tokens, i=dim_in, j=dim_hid, o=dim_in):
  mm1/mm3: psum[j,t] += lhsT=w{1,3}[i_chunk, j_chunk] (stationary, natural
           layout), rhs=xT[i_chunk, t_block] (moving, 512 wide) -> h1T/h3T.
  SwiGLU:  hT = silu(h1T) * h3T  (ACT Silu -> s1, DVE mul fuses PSUM evict).
  mm2:     lhsT=hT[j_chunk, t_chunk] (stationary), rhs=w2[j_chunk, o_half]
           (moving, 512 wide) -> psum[t,o] natural-layout fp32 output.

A single 8-slot PSUM pool rotates banks through mm1/mm3/mm2 accumulators.
Startup queue ordering: x block 0 (DMA + DVE convert + XBAR transpose)
first, then w1 converts (ACT+DVE alternating), then w3 (DVE); w2 converts
are emitted inside block 0 after the mm1 loop so block 0's silus are not
queued behind the w2 DMA wait.
"""

import sys

sys.path.insert(0, "/opt/trn_rl_repo")

import numpy as np

N_CORES = 8
D = 1024  # dim_in
H = 1024  # dim_hid
P = 128
TB = 512  # token block per pipeline stage

_CACHE = {}


def _build(tok):
    import concourse.bacc as bacc
    import concourse.tile as tile
    from concourse import mybir

    dt = mybir.dt
    AF = mybir.ActivationFunctionType
    f32 = dt.float32
    bf = dt.bfloat16

    assert tok % TB == 0
    n_blk = tok // TB
    n_tc = TB // P  # 4 token chunks of 128 per block
    n_i = D // P    # 8 contraction chunks for mm1/mm3
    n_j = H // P    # 8 contraction chunks for mm2

    nc = bacc.Bacc(trn_type="TRN2", target_bir_lowering=False)
    x_h = nc.dram_tensor("x", [tok, D], f32, kind="ExternalInput")
    w1_h = nc.dram_tensor("w1", [D, H], f32, kind="ExternalInput")
    w2_h = nc.dram_tensor("w2", [H, D], f32, kind="ExternalInput")
    w3_h = nc.dram_tensor("w3", [D, H], f32, kind="ExternalInput")
    out_h = nc.dram_tensor("out", [tok, D], f32, kind="ExternalOutput")

    with tile.TileContext(nc) as tc:
        with (
            tc.tile_pool(name="wpool", bufs=1) as wpool,
            tc.tile_pool(name="wstp", bufs=3) as wstp,
            tc.tile_pool(name="xfpool", bufs=2) as xfpool,
            tc.tile_pool(name="xbpool", bufs=2) as xbpool,
            tc.tile_pool(name="xtpool", bufs=2) as xtpool,
            tc.tile_pool(name="htpool", bufs=2) as htpool,
            tc.tile_pool(name="spool", bufs=8) as spool,
            tc.tile_pool(name="opool", bufs=3) as opool,
            tc.tile_pool(name="psum", bufs=8, space="PSUM") as pp,
        ):
            x_r = x_h[:, :].rearrange("(b c p) d -> b p c d", p=P, c=n_tc)
            o_r = out_h[:, :].rearrange("(b c p) d -> b c p d", p=P, c=n_tc)

            x_tiles = {}

            def load_x_block(b):
                xf = xfpool.tile([P, n_tc, D], f32, name="xf", tag="xf")
                nc.sync.dma_start(out=xf, in_=x_r[b])
                xb = xbpool.tile([P, n_tc * D], bf, name="xb", tag="xb")
                nc.vector.tensor_copy(xb, xf.rearrange("p c d -> p (c d)"))
                xT = xtpool.tile([P, n_tc * n_i, P], bf, name="xT", tag="xT")
                nc.sync.dma_start(out=xT, in_=xb, transpose=True)
                # xT[p, tc*n_i + i, t] = x[tc*128 + t, i*128 + p]
                x_tiles[b] = xT.rearrange("p (c i) t -> p i c t", i=n_i)

            # x block 0 first, then w1 (needed first), then w3; w2 converts
            # are deferred into block 0 (after the mm1 loop).
            load_x_block(0)
            w1b = [wpool.tile([P, H], bf, name=f"w1b{i}") for i in range(n_i)]
            w3b = [wpool.tile([P, H], bf, name=f"w3b{i}") for i in range(n_i)]
            w2b = [wpool.tile([P, D], bf, name=f"w2b{j}") for j in range(n_j)]
            for i in range(n_i):
                wst = wstp.tile([P, H], f32, name="wst", tag="wst")
                nc.sync.dma_start(out=wst, in_=w1_h[i * P:(i + 1) * P, :])
                if i % 2 == 0:
                    nc.vector.tensor_copy(w1b[i], wst)
                else:
                    nc.scalar.activation(w1b[i], wst, AF.Copy)
            for i in range(n_i):
                wst = wstp.tile([P, H], f32, name="wst", tag="wst")
                nc.sync.dma_start(out=wst, in_=w3_h[i * P:(i + 1) * P, :])
                nc.vector.tensor_copy(w3b[i], wst)
            w2st = []
            for j in range(n_j):
                wst = wstp.tile([P, D], f32, name="wst", tag="wst")
                nc.sync.dma_start(out=wst, in_=w2_h[j * P:(j + 1) * P, :])
                w2st.append(wst)

            for b in range(n_blk):
                if b + 1 < n_blk:
                    load_x_block(b + 1)
                xT = x_tiles.pop(b)

                # ---- mm1 (j loop; only needs w1) -> s1[j] = silu(x @ w1)
                s1s = []
                for j in range(n_j):
                    pA = pp.tile([P, TB], f32, name="pA", tag="ps")
                    for i in range(n_i):
                        nc.tensor.matmul(
                            pA, w1b[i][:, j * P:(j + 1) * P], xT[:, i],
                            start=(i == 0), stop=(i == n_i - 1),
                        )
                    s1 = spool.tile([P, TB], bf, name="s1", tag="s1")
                    nc.scalar.activation(s1, pA, AF.Silu)
                    s1s.append(s1)

                if b == 0:
                    # w2 converts: emitted after block 0's silus so they don't
                    # block them on the scalar queue; ready before mm2(b0).
                    for j in range(n_j):
                        nc.scalar.activation(w2b[j], w2st[j], AF.Copy)
                    w2st = None

                # ---- mm3 (j loop; needs w3) -> hT[j] = s1[j] * (x @ w3)
                hT = htpool.tile([P, n_j, TB], bf, name="hT")
                for j in range(n_j):
                    pB = pp.tile([P, TB], f32, name="pB", tag="ps")
                    for i in range(n_i):
                        nc.tensor.matmul(
                            pB, w3b[i][:, j * P:(j + 1) * P], xT[:, i],
                            start=(i == 0), stop=(i == n_i - 1),
                        )
                    nc.vector.tensor_mul(hT[:, j, :], pB, s1s[j])

                # ---- mm2 -> natural-layout out, two 512-col halves
                for t in range(n_tc):
                    pC = pp.tile([P, 512], f32, name="pC", tag="ps")
                    for j in range(n_j):
                        nc.tensor.matmul(
                            pC, hT[:, j, t * P:(t + 1) * P], w2b[j][:, 0:512],
                            start=(j == 0), stop=(j == n_j - 1),
                        )
                    pD = pp.tile([P, 512], f32, name="pD", tag="ps")
                    for j in range(n_j):
                        nc.tensor.matmul(
                            pD, hT[:, j, t * P:(t + 1) * P], w2b[j][:, 512:1024],
                            start=(j == 0), stop=(j == n_j - 1),
                        )
                    o_t = opool.tile([P, D], f32, name="o_t", tag="o_t")
                    nc.scalar.activation(o_t[:, 0:512], pC, AF.Copy)
                    nc.vector.tensor_copy(o_t[:, 512:1024], pD)
                    nc.sync.dma_start(out=o_r[b, t], in_=o_t)

    nc.compile()
    return nc


def _get_nc(tok):
    if tok not in _CACHE:
        _CACHE[tok] = _build(tok)
    return _CACHE[tok]


def kernel(x, w1, w2, w3, m_sizes):
    from concourse.bass_utils import run_bass_kernel_spmd

    x = np.asarray(x, dtype=np.float32)
    w1 = np.asarray(w1, dtype=np.float32)
    w2 = np.asarray(w2, dtype=np.float32)
    w3 = np.asarray(w3, dtype=np.float32)
    sizes = np.asarray(m_sizes).astype(np.int64)
    offs = np.concatenate([[0], np.cumsum(sizes)])
    n_exp = sizes.shape[0]
    assert n_exp == N_CORES

    pad = int(max(int(sizes.max()), TB))
    pad = ((pad + TB - 1) // TB) * TB
    nc = _get_nc(pad)

    in_maps = []
    for e in range(N_CORES):
        xe = x[offs[e]:offs[e + 1]]
        if xe.shape[0] < pad:
            xe = np.concatenate(
                [xe, np.zeros((pad - xe.shape[0], D), dtype=np.float32)], axis=0
            )
        in_maps.append({"x": xe, "w1": w1[e], "w2": w2[e], "w3": w3[e]})

    r = run_bass_kernel_spmd(nc, in_maps, core_ids=list(range(N_CORES)))
    out = np.concatenate(
        [r.results[e]["out"][: sizes[e]] for e in range(N_CORES)], axis=0
    )
    return out.astype(np.float32)
